# revision 14
# baseline (speedup 1.0000x reference)
"""ODConv2d Trainium2 kernel.

Data-parallel over batch: 32 samples -> 8 NeuronCores x 4 samples.
Per sample on-device:
  GAP (ACT copy+accum, also launders the x tile onto ACT) -> attention
  trunk -> 4 heads (ch/fl/sp/kn; biases folded in as accumulating
  matmuls with constant operands) -> dynamic weight aggregation on the
  PE (stacked-identity lhsT contracts over the 4 kernel experts; col
  tiling routes each 32-channel group to its PSUM partitions) -> 3x3
  conv as 18 accumulated shift-matmuls over a zero-padded x image in
  bf16 -> fl-scaled fp32 output.

Engine discipline: every matmul operand is produced by ACT (or is a
DMA'd constant pre-touched by a dummy matmul) so that fp32 self-loading
matmuls never need more than one semaphore wait (walrus S3_LW limit).

All shapes hardcoded for B=32, C=O=256, H=W=56, K=4, A=16, k=3.
"""

import numpy as np

import concourse.bass as bass
import concourse.bacc as bacc
import concourse.mybir as mybir
import concourse.tile as tile
from concourse.bass_utils import run_bass_kernel_spmd

F32 = mybir.dt.float32
BF16 = mybir.dt.bfloat16
AF = mybir.ActivationFunctionType

NCORES = 8
B, C, H, W = 32, 256, 56, 56
O, K, KK, A = 256, 4, 3, 16
BL = B // NCORES          # samples per core
HW = H * W                # 3136
PH, PW = H + 2, W + 2     # 58
PHW = PH * PW             # 3364
EPS = 1e-5
TEMP = 1.0
NT = 7                    # output row-tiles per sample (8 rows x 56 cols)
ROWS = H // NT            # 8
NFREE = ROWS * W          # 448
GO = 9 * O                # 2304: aggregated-weight free size per C-tile

# tiny-psum region columns (single [128, 289] tile per sample)
R_APS = 0          # a_ps        [16, 1]
R_HROW = 1         # head logits [1, 265]
R_KNL = 266        # kn logits   [4, 1]
R_SSUM = 267       # sum(exp)    [1, 1]
R_KNB = 268        # kn bcast    [128, 1]
R_CS = 269         # chsp        [128, 9] x2
R_FL = 287         # fl logits   [128, 1] x2
R_KNLR = 289       # kn logits row form [1, 4] (dve mode)
R_KNB4 = 293       # kn broadcast [128, 4]    (dve mode)
TINY_COLS = 297

# bias-row columns in the brow constant
BB_BETA = 0        # bn beta     [16]
BB_HEAD = 16       # ch/sp/kn    [269]
BB_FL = 285        # fl bias     [256]
BROW_COLS = 541


def _build_nc(loop_r=None, depth=1, xp_bufs=4, sm_bufs=2, agg_bufs=2, aps_bufs=3, osb_bufs=3, nb=BL, agg_mode="dve", agg_tiled=False):
    nc = bacc.Bacc()

    xpad = nc.dram_tensor("xpad", [BL * C, PHW], BF16, kind="ExternalInput")
    w5 = nc.dram_tensor("w5", [128, 8 * GO], BF16, kind="ExternalInput")
    w6 = nc.dram_tensor("w6", [C, 4 * GO], BF16, kind="ExternalInput")
    onesr = nc.dram_tensor("onesr", [1, 128], F32, kind="ExternalInput")
    fcw = nc.dram_tensor("fcw", [128, 32], F32, kind="ExternalInput")
    headsw = nc.dram_tensor("headsw", [16, 269], F32, kind="ExternalInput")
    flw = nc.dram_tensor("flw", [16, 256], F32, kind="ExternalInput")
    brow = nc.dram_tensor("brow", [1, BROW_COLS], F32, kind="ExternalInput")
    cmask = nc.dram_tensor("cmask", [128, 32], F32, kind="ExternalInput")
    cp4 = nc.dram_tensor("cp4", [4, 128], F32, kind="ExternalInput")
    ones4 = nc.dram_tensor("ones4", [4, 1], F32, kind="ExternalInput")
    out = nc.dram_tensor("out", [BL * C, HW], F32, kind="ExternalOutput")

    with tile.TileContext(nc) as tc:
        with (
            tc.tile_pool(name="cw", bufs=1) as cw_pool,
            tc.tile_pool(name="cs", bufs=1) as cs_pool,
            tc.tile_pool(name="xp", bufs=xp_bufs) as xp_pool,
            tc.tile_pool(name="agg", bufs=agg_bufs) as agg_pool,
            tc.tile_pool(name="osb", bufs=osb_bufs) as osb_pool,
            tc.tile_pool(name="sm", bufs=sm_bufs) as sm_pool,
            tc.tile_pool(name="acc", bufs=2) as acc_pool,
            tc.tile_pool(name="aps", bufs=aps_bufs, space="PSUM") as aps_pool,
            tc.tile_pool(name="cps", bufs=2, space="PSUM") as cps_pool,
            tc.tile_pool(name="tps", bufs=2, space="PSUM") as tps_pool,
        ):
            # --- resident constants ---
            w5_sb = None
            w6_sb = None
            onesr_sb = None
            if agg_mode == "dve":
                w6_sb = []
                for t in range(2):
                    w6t = cw_pool.tile([128, 4 * GO], BF16, name=f"w6_sb{t}",
                                       tag=f"w6_{t}")
                    for k in range(4):
                        nc.sync.dma_start(
                            w6t[:, k * GO : (k + 1) * GO],
                            w6[t * 128 : (t + 1) * 128, k * GO : (k + 1) * GO])
                    w6_sb.append(w6t)
                onesr_sb = cs_pool.tile([1, 128], F32, name="onesr_sb")
                nc.sync.dma_start(onesr_sb[:], onesr[:])
            else:
                w5_sb = cw_pool.tile([128, 8 * GO], BF16, name="w5_sb")
                for g in range(8):
                    nc.sync.dma_start(w5_sb[:, g * GO : (g + 1) * GO],
                                      w5[:, g * GO : (g + 1) * GO])
            fcw_sb = cs_pool.tile([128, 32], F32, name="fcw_sb")
            nc.sync.dma_start(fcw_sb[:], fcw[:])
            headsw_sb = cs_pool.tile([16, 269], F32, name="headsw_sb")
            nc.sync.dma_start(headsw_sb[:], headsw[:])
            flw_sb = cs_pool.tile([16, 256], F32, name="flw_sb")
            nc.sync.dma_start(flw_sb[:], flw[:])
            brow_sb = cs_pool.tile([1, BROW_COLS], F32, name="brow_sb")
            nc.sync.dma_start(brow_sb[:], brow[:])
            cmask_sb = cp4_sb = None
            if agg_mode != "dve":
                cmask_sb = cs_pool.tile([128, 32], F32, name="cmask_sb")
                nc.sync.dma_start(cmask_sb[:], cmask[:])
                cp4_sb = cs_pool.tile([4, 128], F32, name="cp4_sb")
                nc.sync.dma_start(cp4_sb[:], cp4[:])
            ones4_sb = cs_pool.tile([4, 1], F32, name="ones4_sb")
            nc.sync.dma_start(ones4_sb[:], ones4[:])
            one_sb = ones4_sb[0:1, 0:1]

            # pre-touch every PE-read constant so later matmuls never carry
            # a DMA wait on top of a data wait
            trash = tps_pool.tile([128, 16], F32, name="trash", tag="trash", bufs=1)
            touches = [fcw_sb[:, 0:1], headsw_sb[0:16, 0:1], flw_sb[0:16, 0:1],
                       brow_sb[0:1, 0:1], ones4_sb[0:4, 0:1]]
            if agg_mode != "dve":
                touches += [cp4_sb[0:4, 0:1]]
            else:
                touches += [onesr_sb[0:1, 0:1]]
            for lhsT in touches:
                nc.tensor.matmul(trash[0 : lhsT.shape[1], 0:1], lhsT, lhsT)
            if agg_mode != "dve":
                for g in range(8):
                    nc.tensor.matmul(trash[0:1, 0:1], w5_sb[:, g * GO : g * GO + 1],
                                     w5_sb[:, g * GO : g * GO + 1])

            state = {}

            def prep(b):
                st = {}
                # x load (pre-padded bf16; borders stay zero)
                xp = []
                for t in range(2):
                    xt = xp_pool.tile([128, PHW], BF16, name=f"xp{b}_{t}", tag="xp")
                    nc.sync.dma_start(
                        xt[:], xpad[b * C + t * 128 : b * C + (t + 1) * 128, :]
                    )
                    xp.append(xt)
                st["xp"] = xp
                # GAP on ACT: in-place copy + free-dim accumulate.  Also makes
                # ACT the last writer of xp so conv matmuls wait only on ACT.
                s2 = sm_pool.tile([128, 2], F32, name=f"s2_{b}", tag="s2")
                for t in range(2):
                    nc.scalar.activation(xp[t][:], xp[t][:], AF.Copy,
                                         accum_out=s2[:, t : t + 1])
                tiny = tps_pool.tile([128, TINY_COLS], F32, name=f"tiny{b}", tag="tiny")
                # attention trunk: a = relu(fcw.T @ s + beta)
                a_ps = tiny[0:16, R_APS : R_APS + 1]
                for t in range(2):
                    nc.tensor.matmul(a_ps, fcw_sb[:, 16 * t : 16 * t + 16],
                                     s2[:, t : t + 1], start=(t == 0), stop=False)
                nc.tensor.matmul(a_ps, brow_sb[0:1, BB_BETA : BB_BETA + 16], one_sb,
                                 start=False, stop=True)
                a_col = sm_pool.tile([16, 1], F32, name=f"a_col{b}", tag="a_col")
                nc.scalar.activation(a_col[:], a_ps, AF.Relu)
                # head logits (row form): ch [0:256), sp [256:265)
                hrow = tiny[0:1, R_HROW : R_HROW + 265]
                nc.tensor.matmul(hrow, a_col[:], headsw_sb[0:16, 0:265],
                                 start=True, stop=False)
                nc.tensor.matmul(hrow, one_sb, brow_sb[0:1, BB_HEAD : BB_HEAD + 265],
                                 start=False, stop=True)
                ch_row = sm_pool.tile([1, 256], F32, name=f"ch_row{b}", tag="ch_row")
                nc.scalar.activation(ch_row[:], tiny[0:1, R_HROW : R_HROW + 256],
                                     AF.Sigmoid)
                sp_row = sm_pool.tile([1, 9], F32, name=f"sp_row{b}", tag="sp_row")
                nc.scalar.activation(sp_row[:], tiny[0:1, R_HROW + 256 : R_HROW + 265],
                                     AF.Sigmoid)
                # kernel-attention softmax
                if agg_mode == "dve":
                    knlr = tiny[0:1, R_KNLR : R_KNLR + 4]
                    nc.tensor.matmul(knlr, a_col[:], headsw_sb[0:16, 265:269],
                                     start=True, stop=False)
                    nc.tensor.matmul(knlr, one_sb,
                                     brow_sb[0:1, BB_HEAD + 265 : BB_HEAD + 269],
                                     start=False, stop=True)
                    expr = sm_pool.tile([1, 4], F32, name=f"expr{b}", tag="expr")
                    nc.scalar.activation(expr[:], knlr, AF.Exp)
                    ssr = sm_pool.tile([1, 1], F32, name=f"ssr{b}", tag="ssr")
                    nc.vector.reduce_sum(ssr[:], expr[:], axis=mybir.AxisListType.X)
                    rsc = sm_pool.tile([1, 1], F32, name=f"rsc{b}", tag="rsc")
                    nc.vector.reciprocal(rsc[:], ssr[:])
                    chrp = sm_pool.tile([1, 256], F32, name=f"chrp{b}", tag="chrp")
                    nc.scalar.activation(chrp[:], ch_row[:], AF.Copy, scale=rsc[:])
                    # kn broadcast to all partitions: [128,4] = ones128 (x) expr
                    nc.tensor.matmul(tiny[0:128, R_KNB4 : R_KNB4 + 4], onesr_sb[:],
                                     expr[:])
                    knb4 = sm_pool.tile([128, 4], F32, name=f"knb4{b}", tag="knb4")
                    nc.scalar.activation(knb4[:], tiny[0:128, R_KNB4 : R_KNB4 + 4],
                                         AF.Copy)
                    stripe = None
                else:
                    knl = tiny[0:4, R_KNL : R_KNL + 1]
                    nc.tensor.matmul(knl, headsw_sb[0:16, 265:269], a_col[:],
                                     start=True, stop=False)
                    nc.tensor.matmul(knl, brow_sb[0:1, BB_HEAD + 265 : BB_HEAD + 269],
                                     one_sb, start=False, stop=True)
                    expc = sm_pool.tile([4, 1], F32, name=f"expc{b}", tag="expc")
                    nc.scalar.activation(expc[:], knl, AF.Exp)
                    nc.tensor.matmul(tiny[0:1, R_SSUM : R_SSUM + 1], expc[:], ones4_sb[:])
                    rsc = sm_pool.tile([1, 1], F32, name=f"rsc{b}", tag="rsc")
                    nc.vector.reciprocal(rsc[:], tiny[0:1, R_SSUM : R_SSUM + 1])
                    chrp = sm_pool.tile([1, 256], F32, name=f"chrp{b}", tag="chrp")
                    nc.scalar.activation(chrp[:], ch_row[:], AF.Copy, scale=rsc[:])
                    nc.tensor.matmul(tiny[0:128, R_KNB : R_KNB + 1], cp4_sb[:], expc[:])
                    knb = sm_pool.tile([128, 1], F32, name=f"knb{b}", tag="knb")
                    nc.scalar.activation(knb[:], tiny[0:128, R_KNB : R_KNB + 1], AF.Copy)
                    stripe = sm_pool.tile([128, 32], BF16, name=f"stripe{b}", tag="stripe")
                    nc.scalar.activation(stripe[:], cmask_sb[:], AF.Copy, scale=knb[:])
                # chsp[c, ij] = ch'[c] * sp[ij]  (outer product per C-tile)
                chsp = sm_pool.tile([128, 18], F32, name=f"chsp{b}", tag="chsp")
                for t in range(2):
                    cs_ps = tiny[0:128, R_CS + 9 * t : R_CS + 9 * t + 9]
                    nc.tensor.matmul(cs_ps, chrp[0:1, 128 * t : 128 * t + 128],
                                     sp_row[:])
                    nc.vector.tensor_copy(chsp[:, 9 * t : 9 * t + 9], cs_ps)
                # fl head (col form, per O-tile)
                fl = sm_pool.tile([128, 2], F32, name=f"fl{b}", tag="fl")
                for t in range(2):
                    fl_ps = tiny[0:128, R_FL + t : R_FL + t + 1]
                    nc.tensor.matmul(fl_ps, flw_sb[0:16, 128 * t : 128 * t + 128],
                                     a_col[:], start=True, stop=False)
                    nc.tensor.matmul(fl_ps,
                                     brow_sb[0:1, BB_FL + 128 * t : BB_FL + 128 * t + 128],
                                     one_sb, start=False, stop=True)
                    nc.scalar.activation(fl[:, t : t + 1], fl_ps, AF.Sigmoid)
                st["fl"] = fl
                # weight aggregation: agg = (sum_k kn[k] * w[k]) * chsp
                aggT = []
                if agg_mode == "dve":
                    for t in range(2):
                        at = agg_pool.tile([128, GO], BF16, name=f"aggT{b}_{t}",
                                           tag=f"agg{t}")
                        acc = acc_pool.tile([128, GO], BF16, name=f"acc{b}_{t}",
                                            tag="acca")
                        nc.vector.tensor_scalar_mul(acc[:], w6_sb[t][:, 0:GO],
                                                    knb4[:, 0:1])
                        for k in range(1, 4):
                            nc.vector.scalar_tensor_tensor(
                                acc[:], w6_sb[t][:, k * GO : (k + 1) * GO],
                                knb4[:, k : k + 1], acc[:],
                                op0=mybir.AluOpType.mult, op1=mybir.AluOpType.add)
                        for ij in range(9):
                            nc.vector.tensor_scalar_mul(
                                at[:, ij * 256 : (ij + 1) * 256],
                                acc[:, ij * 256 : (ij + 1) * 256],
                                chsp[:, 9 * t + ij : 9 * t + ij + 1])
                        aggT.append(at)
                    st["aggT"] = aggT
                    state[b] = st
                    return
                bounds = [(0, 512), (512, 1024), (1024, 1536), (1536, 2048),
                          (2048, 2304)]
                for t in range(2):
                    at = agg_pool.tile([128, GO], BF16, name=f"aggT{b}_{t}",
                                       tag=f"agg{t}")
                    for (c0, c1) in bounds:
                        n = c1 - c0
                        aps = aps_pool.tile([128, 512], F32,
                                            name=f"aps{b}_{t}_{c0}", tag="aps")
                        # 16 concurrent 32x32 tiles: row-group i holds expert
                        # i's weights, col-group j accumulates channel group
                        # 4t+j; stripe[32i:32i+32] is kn[i]*I32.
                        for j in range(4):
                            g0 = (4 * t + j) * GO
                            if agg_tiled:
                                for i in range(4):
                                    nc.tensor.matmul(
                                        aps[32 * j : 32 * j + 32, 0:n],
                                        stripe[32 * i : 32 * i + 32, :],
                                        w5_sb[32 * i : 32 * i + 32, g0 + c0 : g0 + c1],
                                        tile_position=(32 * i, 32 * j),
                                        start=(i == 0), stop=(i == 3),
                                    )
                            else:
                                nc.tensor.matmul(
                                    aps[32 * j : 32 * j + 32, 0:n],
                                    stripe[:],
                                    w5_sb[:, g0 + c0 : g0 + c1],
                                    tile_position=(0, 32 * j),
                                )
                        for ij in range(c0 // 256, c1 // 256):
                            nc.vector.tensor_scalar_mul(
                                at[:, ij * 256 : (ij + 1) * 256],
                                aps[:, ij * 256 - c0 : (ij + 1) * 256 - c0],
                                chsp[:, 9 * t + ij : 9 * t + ij + 1],
                            )
                    aggT.append(at)
                st["aggT"] = aggT
                state[b] = st

            def conv(b):
                st = state[b]
                xv = [st["xp"][t][:].rearrange("p (h w) -> p h w", w=PW)
                      for t in range(2)]
                for ot in range(2):
                    for nt in range(NT):
                        cps = cps_pool.tile([128, NFREE], F32,
                                            name=f"cps{b}_{ot}_{nt}", tag="cps")
                        idx = 0
                        for t in range(2):
                            for ij in range(9):
                                i, jj = divmod(ij, 3)
                                nc.tensor.matmul(
                                    cps[:],
                                    st["aggT"][t][:, ij * 256 + ot * 128 :
                                                  ij * 256 + ot * 128 + 128],
                                    xv[t][:, ROWS * nt + i : ROWS * nt + i + ROWS,
                                          jj : jj + W],
                                    start=(idx == 0), stop=(idx == 17),
                                )
                                idx += 1
                        osb = osb_pool.tile([128, NFREE], F32,
                                            name=f"osb{b}_{ot}_{nt}", tag="osb")
                        nc.scalar.activation(osb[:], cps[:], AF.Copy,
                                             scale=st["fl"][:, ot : ot + 1])
                        nc.sync.dma_start(
                            out[b * C + ot * 128 : b * C + ot * 128 + 128,
                                nt * NFREE : (nt + 1) * NFREE],
                            osb[:],
                        )
                del state[b]

            def body():
                for b in range(depth):
                    prep(b)
                for b in range(depth, nb):
                    prep(b)
                    conv(b - depth)
                for b in range(nb - depth, nb):
                    conv(b)

            if loop_r is None:
                body()
            else:
                with tc.For_i(0, loop_r, 1):
                    body()

    if not nc.is_finalized():
        nc.finalize()
    return nc


_NC_CACHE = None


def _get_nc(loop_r=None):
    global _NC_CACHE
    if loop_r is not None:
        return _build_nc(loop_r)
    if _NC_CACHE is None:
        _NC_CACHE = _build_nc()
    return _NC_CACHE


def _host_prep(x, weight, fc_w, bn_gamma, bn_beta, ch_w, ch_b, fl_w, fl_b,
               sp_w, sp_b, kn_w, kn_b):
    import ml_dtypes
    f = np.float32
    bf = ml_dtypes.bfloat16

    x = np.ascontiguousarray(x, dtype=f)
    xpad = np.zeros((B, C, PH, PW), dtype=bf)
    xpad[:, :, 1:-1, 1:-1] = x.astype(bf)
    xpad = xpad.reshape(B, C, PHW)

    # W5[(k,c32), g, ij*O+o] = weight[k, o, g*32+c32, ij]
    wt = np.asarray(weight, dtype=f).reshape(K, O, 8, 32, 9)  # k o g c32 ij
    w5 = np.ascontiguousarray(
        wt.transpose(0, 3, 2, 4, 1).reshape(128, 8 * GO).astype(bf)
    )
    # W6[c, k, ij*O+o] = weight[k, o, c, ij]
    w6 = np.ascontiguousarray(
        np.asarray(weight, dtype=f).reshape(K, O, C, 9)
        .transpose(2, 0, 3, 1).reshape(C, 4 * GO).astype(bf)
    )
    onesr = np.ones((1, 128), dtype=f)

    g16 = np.asarray(bn_gamma, dtype=f) / np.sqrt(f(1.0) + f(EPS))
    fc_w2 = (np.asarray(fc_w, dtype=f) * g16[:, None] / f(HW)).T  # [256,16]
    fcw = np.ascontiguousarray(np.concatenate([fc_w2[:128], fc_w2[128:]], axis=1))

    it = f(1.0 / TEMP)
    headsw = np.zeros((16, 269), dtype=f)
    headsw[:, 0:256] = np.asarray(ch_w, f).T * it
    headsw[:, 256:265] = np.asarray(sp_w, f).T * it
    headsw[:, 265:269] = np.asarray(kn_w, f).T * it
    flw = np.ascontiguousarray(np.asarray(fl_w, f).T * it)

    brow = np.zeros((1, BROW_COLS), dtype=f)
    brow[0, BB_BETA : BB_BETA + 16] = np.asarray(bn_beta, f)
    brow[0, BB_HEAD : BB_HEAD + 256] = np.asarray(ch_b, f) * it
    brow[0, BB_HEAD + 256 : BB_HEAD + 265] = np.asarray(sp_b, f) * it
    brow[0, BB_HEAD + 265 : BB_HEAD + 269] = np.asarray(kn_b, f) * it
    brow[0, BB_FL : BB_FL + 256] = np.asarray(fl_b, f) * it

    cmask = np.ascontiguousarray(np.tile(np.eye(32, dtype=f), (4, 1)))
    cp4 = np.ascontiguousarray(np.repeat(np.eye(4, dtype=f), 32, axis=1))
    ones4 = np.ones((4, 1), dtype=f)

    shared = dict(w5=w5, w6=w6, onesr=onesr, fcw=fcw, headsw=headsw,
                  flw=flw, brow=brow, cmask=cmask, cp4=cp4, ones4=ones4)
    in_maps = []
    for ci in range(NCORES):
        m = dict(shared)
        m["xpad"] = np.ascontiguousarray(
            xpad[ci * BL : (ci + 1) * BL].reshape(BL * C, PHW)
        )
        in_maps.append(m)
    return in_maps


def kernel(**inputs):
    nc = _get_nc()
    in_maps = _host_prep(**inputs)
    res = run_bass_kernel_spmd(nc, in_maps, list(range(NCORES)))
    outs = [res.results[i]["out"].reshape(BL, C, H, W) for i in range(NCORES)]
    return np.concatenate(outs, axis=0)


if __name__ == "__main__":
    nc = _get_nc()
    print("built ok")


# revision 15
# speedup vs baseline: 1.1233x; 1.1233x over previous
"""ODConv2d Trainium2 kernel.

Data-parallel over batch: 32 samples -> 8 NeuronCores x 4 samples.
Per sample on-device:
  GAP (ACT copy+accum, also launders the x tile onto ACT) -> attention
  trunk -> 4 heads (ch/fl/sp/kn; biases folded in as accumulating
  matmuls with constant operands) -> dynamic weight aggregation on the
  PE (stacked-identity lhsT contracts over the 4 kernel experts; col
  tiling routes each 32-channel group to its PSUM partitions) -> 3x3
  conv as 18 accumulated shift-matmuls over a zero-padded x image in
  bf16 -> fl-scaled fp32 output.

Engine discipline: every matmul operand is produced by ACT (or is a
DMA'd constant pre-touched by a dummy matmul) so that fp32 self-loading
matmuls never need more than one semaphore wait (walrus S3_LW limit).

All shapes hardcoded for B=32, C=O=256, H=W=56, K=4, A=16, k=3.
"""

import numpy as np

import concourse.bass as bass
import concourse.bacc as bacc
import concourse.mybir as mybir
import concourse.tile as tile
from concourse.bass_utils import run_bass_kernel_spmd

F32 = mybir.dt.float32
BF16 = mybir.dt.bfloat16
AF = mybir.ActivationFunctionType

NCORES = 8
B, C, H, W = 32, 256, 56, 56
O, K, KK, A = 256, 4, 3, 16
BL = B // NCORES          # samples per core
HW = H * W                # 3136
PH, PW = H + 2, W + 2     # 58
PHW = PH * PW             # 3364
EPS = 1e-5
TEMP = 1.0
NT = 7                    # output row-tiles per sample (8 rows x 56 cols)
ROWS = H // NT            # 8
NFREE = ROWS * W          # 448
GO = 9 * O                # 2304: aggregated-weight free size per C-tile

# tiny-psum region columns (single [128, 289] tile per sample)
R_APS = 0          # a_ps        [16, 1]
R_HROW = 1         # head logits [1, 265]
R_KNL = 266        # kn logits   [4, 1]
R_SSUM = 267       # sum(exp)    [1, 1]
R_KNB = 268        # kn bcast    [128, 1]
R_CS = 269         # chsp        [128, 9] x2
R_FL = 287         # fl logits   [128, 1] x2
R_KNLR = 289       # kn logits row form [1, 4] (dve mode)
R_KNB4 = 293       # kn broadcast [128, 4]    (dve mode)
TINY_COLS = 297

# bias-row columns in the brow constant
BB_BETA = 0        # bn beta     [16]
BB_HEAD = 16       # ch/sp/kn    [269]
BB_FL = 285        # fl bias     [256]
BROW_COLS = 541


def _build_nc(loop_r=None, depth=1, xp_bufs=4, sm_bufs=2, agg_bufs=2, aps_bufs=3, osb_bufs=3, nb=BL, agg_mode="dve", agg_tiled=False):
    nc = bacc.Bacc()

    xpad = nc.dram_tensor("xpad", [BL * C, PHW], BF16, kind="ExternalInput")
    w5 = nc.dram_tensor("w5", [128, 8 * GO], BF16, kind="ExternalInput")
    w6 = nc.dram_tensor("w6", [C, 4 * GO], BF16, kind="ExternalInput")
    onesr = nc.dram_tensor("onesr", [1, 128], F32, kind="ExternalInput")
    fcw = nc.dram_tensor("fcw", [128, 32], F32, kind="ExternalInput")
    headsw = nc.dram_tensor("headsw", [16, 269], F32, kind="ExternalInput")
    flw = nc.dram_tensor("flw", [16, 256], F32, kind="ExternalInput")
    brow = nc.dram_tensor("brow", [1, BROW_COLS], F32, kind="ExternalInput")
    cmask = nc.dram_tensor("cmask", [128, 32], F32, kind="ExternalInput")
    cp4 = nc.dram_tensor("cp4", [4, 128], F32, kind="ExternalInput")
    ones4 = nc.dram_tensor("ones4", [4, 1], F32, kind="ExternalInput")
    out = nc.dram_tensor("out", [BL * C, HW], F32, kind="ExternalOutput")

    with tile.TileContext(nc) as tc:
        with (
            tc.tile_pool(name="cw", bufs=1) as cw_pool,
            tc.tile_pool(name="cs", bufs=1) as cs_pool,
            tc.tile_pool(name="xp", bufs=xp_bufs) as xp_pool,
            tc.tile_pool(name="agg", bufs=agg_bufs) as agg_pool,
            tc.tile_pool(name="osb", bufs=osb_bufs) as osb_pool,
            tc.tile_pool(name="sm", bufs=sm_bufs) as sm_pool,
            tc.tile_pool(name="acc", bufs=2) as acc_pool,
            tc.tile_pool(name="aps", bufs=aps_bufs, space="PSUM") as aps_pool,
            tc.tile_pool(name="cps", bufs=2, space="PSUM") as cps_pool,
            tc.tile_pool(name="tps", bufs=2, space="PSUM") as tps_pool,
        ):
            # --- resident constants ---
            w5_sb = None
            w6_sb = None
            onesr_sb = None
            if agg_mode == "dve":
                w6_sb = []
                for t in range(2):
                    w6t = cw_pool.tile([128, 4 * GO], BF16, name=f"w6_sb{t}",
                                       tag=f"w6_{t}")
                    for k in range(4):
                        nc.sync.dma_start(
                            w6t[:, k * GO : (k + 1) * GO],
                            w6[t * 128 : (t + 1) * 128, k * GO : (k + 1) * GO])
                    w6_sb.append(w6t)
                onesr_sb = cs_pool.tile([1, 128], F32, name="onesr_sb")
                nc.sync.dma_start(onesr_sb[:], onesr[:])
            else:
                w5_sb = cw_pool.tile([128, 8 * GO], BF16, name="w5_sb")
                for g in range(8):
                    nc.sync.dma_start(w5_sb[:, g * GO : (g + 1) * GO],
                                      w5[:, g * GO : (g + 1) * GO])
            fcw_sb = cs_pool.tile([128, 32], F32, name="fcw_sb")
            nc.sync.dma_start(fcw_sb[:], fcw[:])
            headsw_sb = cs_pool.tile([16, 269], F32, name="headsw_sb")
            nc.sync.dma_start(headsw_sb[:], headsw[:])
            flw_sb = cs_pool.tile([16, 256], F32, name="flw_sb")
            nc.sync.dma_start(flw_sb[:], flw[:])
            brow_sb = cs_pool.tile([1, BROW_COLS], F32, name="brow_sb")
            nc.sync.dma_start(brow_sb[:], brow[:])
            cmask_sb = cp4_sb = None
            if agg_mode != "dve":
                cmask_sb = cs_pool.tile([128, 32], F32, name="cmask_sb")
                nc.sync.dma_start(cmask_sb[:], cmask[:])
                cp4_sb = cs_pool.tile([4, 128], F32, name="cp4_sb")
                nc.sync.dma_start(cp4_sb[:], cp4[:])
            ones4_sb = cs_pool.tile([4, 1], F32, name="ones4_sb")
            nc.sync.dma_start(ones4_sb[:], ones4[:])
            one_sb = ones4_sb[0:1, 0:1]

            # pre-touch every PE-read constant so later matmuls never carry
            # a DMA wait on top of a data wait
            trash = tps_pool.tile([128, 16], F32, name="trash", tag="trash", bufs=1)
            touches = [fcw_sb[:, 0:1], headsw_sb[0:16, 0:1], flw_sb[0:16, 0:1],
                       brow_sb[0:1, 0:1], ones4_sb[0:4, 0:1]]
            if agg_mode != "dve":
                touches += [cp4_sb[0:4, 0:1]]
            else:
                touches += [onesr_sb[0:1, 0:1]]
            for lhsT in touches:
                nc.tensor.matmul(trash[0 : lhsT.shape[1], 0:1], lhsT, lhsT)
            if agg_mode != "dve":
                for g in range(8):
                    nc.tensor.matmul(trash[0:1, 0:1], w5_sb[:, g * GO : g * GO + 1],
                                     w5_sb[:, g * GO : g * GO + 1])

            state = {}

            def prep(b):
                st = {}
                # x load (pre-padded bf16; borders stay zero)
                xp = []
                for t in range(2):
                    xt = xp_pool.tile([128, PHW], BF16, name=f"xp{b}_{t}", tag="xp")
                    nc.sync.dma_start(
                        xt[:], xpad[b * C + t * 128 : b * C + (t + 1) * 128, :]
                    )
                    xp.append(xt)
                st["xp"] = xp
                # GAP on ACT: in-place copy + free-dim accumulate.  Also makes
                # ACT the last writer of xp so conv matmuls wait only on ACT.
                s2 = sm_pool.tile([128, 2], F32, name=f"s2_{b}", tag="s2")
                for t in range(2):
                    nc.scalar.activation(xp[t][:], xp[t][:], AF.Copy,
                                         accum_out=s2[:, t : t + 1])
                tiny = tps_pool.tile([128, TINY_COLS], F32, name=f"tiny{b}", tag="tiny")
                # attention trunk: a = relu(fcw.T @ s + beta)
                a_ps = tiny[0:16, R_APS : R_APS + 1]
                for t in range(2):
                    nc.tensor.matmul(a_ps, fcw_sb[:, 16 * t : 16 * t + 16],
                                     s2[:, t : t + 1], start=(t == 0), stop=False)
                nc.tensor.matmul(a_ps, brow_sb[0:1, BB_BETA : BB_BETA + 16], one_sb,
                                 start=False, stop=True)
                a_col = sm_pool.tile([16, 1], F32, name=f"a_col{b}", tag="a_col")
                nc.scalar.activation(a_col[:], a_ps, AF.Relu)
                # head logits (row form): ch [0:256), sp [256:265)
                hrow = tiny[0:1, R_HROW : R_HROW + 265]
                nc.tensor.matmul(hrow, a_col[:], headsw_sb[0:16, 0:265],
                                 start=True, stop=False)
                nc.tensor.matmul(hrow, one_sb, brow_sb[0:1, BB_HEAD : BB_HEAD + 265],
                                 start=False, stop=True)
                ch_row = sm_pool.tile([1, 256], F32, name=f"ch_row{b}", tag="ch_row")
                nc.scalar.activation(ch_row[:], tiny[0:1, R_HROW : R_HROW + 256],
                                     AF.Sigmoid)
                sp_row = sm_pool.tile([1, 9], F32, name=f"sp_row{b}", tag="sp_row")
                nc.scalar.activation(sp_row[:], tiny[0:1, R_HROW + 256 : R_HROW + 265],
                                     AF.Sigmoid)
                # kernel-attention softmax
                if agg_mode == "dve":
                    knlr = tiny[0:1, R_KNLR : R_KNLR + 4]
                    nc.tensor.matmul(knlr, a_col[:], headsw_sb[0:16, 265:269],
                                     start=True, stop=False)
                    nc.tensor.matmul(knlr, one_sb,
                                     brow_sb[0:1, BB_HEAD + 265 : BB_HEAD + 269],
                                     start=False, stop=True)
                    expr = sm_pool.tile([1, 4], F32, name=f"expr{b}", tag="expr")
                    nc.scalar.activation(expr[:], knlr, AF.Exp)
                    ssr = sm_pool.tile([1, 1], F32, name=f"ssr{b}", tag="ssr")
                    nc.vector.reduce_sum(ssr[:], expr[:], axis=mybir.AxisListType.X)
                    rsc = sm_pool.tile([1, 1], F32, name=f"rsc{b}", tag="rsc")
                    nc.vector.reciprocal(rsc[:], ssr[:])
                    chrp = sm_pool.tile([1, 256], F32, name=f"chrp{b}", tag="chrp")
                    nc.scalar.activation(chrp[:], ch_row[:], AF.Copy, scale=rsc[:])
                    # kn broadcast to all partitions: [128,4] = ones128 (x) expr
                    nc.tensor.matmul(tiny[0:128, R_KNB4 : R_KNB4 + 4], onesr_sb[:],
                                     expr[:])
                    knb4 = sm_pool.tile([128, 4], F32, name=f"knb4{b}", tag="knb4")
                    nc.scalar.activation(knb4[:], tiny[0:128, R_KNB4 : R_KNB4 + 4],
                                         AF.Copy)
                    stripe = None
                else:
                    knl = tiny[0:4, R_KNL : R_KNL + 1]
                    nc.tensor.matmul(knl, headsw_sb[0:16, 265:269], a_col[:],
                                     start=True, stop=False)
                    nc.tensor.matmul(knl, brow_sb[0:1, BB_HEAD + 265 : BB_HEAD + 269],
                                     one_sb, start=False, stop=True)
                    expc = sm_pool.tile([4, 1], F32, name=f"expc{b}", tag="expc")
                    nc.scalar.activation(expc[:], knl, AF.Exp)
                    nc.tensor.matmul(tiny[0:1, R_SSUM : R_SSUM + 1], expc[:], ones4_sb[:])
                    rsc = sm_pool.tile([1, 1], F32, name=f"rsc{b}", tag="rsc")
                    nc.vector.reciprocal(rsc[:], tiny[0:1, R_SSUM : R_SSUM + 1])
                    chrp = sm_pool.tile([1, 256], F32, name=f"chrp{b}", tag="chrp")
                    nc.scalar.activation(chrp[:], ch_row[:], AF.Copy, scale=rsc[:])
                    nc.tensor.matmul(tiny[0:128, R_KNB : R_KNB + 1], cp4_sb[:], expc[:])
                    knb = sm_pool.tile([128, 1], F32, name=f"knb{b}", tag="knb")
                    nc.scalar.activation(knb[:], tiny[0:128, R_KNB : R_KNB + 1], AF.Copy)
                    stripe = sm_pool.tile([128, 32], BF16, name=f"stripe{b}", tag="stripe")
                    nc.scalar.activation(stripe[:], cmask_sb[:], AF.Copy, scale=knb[:])
                # chsp[c, ij] = ch'[c] * sp[ij]  (outer product per C-tile)
                chsp = sm_pool.tile([128, 18], F32, name=f"chsp{b}", tag="chsp")
                for t in range(2):
                    cs_ps = tiny[0:128, R_CS + 9 * t : R_CS + 9 * t + 9]
                    nc.tensor.matmul(cs_ps, chrp[0:1, 128 * t : 128 * t + 128],
                                     sp_row[:])
                    nc.vector.tensor_copy(chsp[:, 9 * t : 9 * t + 9], cs_ps)
                # fl head (col form, per O-tile)
                fl = sm_pool.tile([128, 2], F32, name=f"fl{b}", tag="fl")
                for t in range(2):
                    fl_ps = tiny[0:128, R_FL + t : R_FL + t + 1]
                    nc.tensor.matmul(fl_ps, flw_sb[0:16, 128 * t : 128 * t + 128],
                                     a_col[:], start=True, stop=False)
                    nc.tensor.matmul(fl_ps,
                                     brow_sb[0:1, BB_FL + 128 * t : BB_FL + 128 * t + 128],
                                     one_sb, start=False, stop=True)
                    nc.scalar.activation(fl[:, t : t + 1], fl_ps, AF.Sigmoid)
                st["fl"] = fl
                # weight aggregation: agg = (sum_k kn[k] * w[k]) * chsp
                aggT = []
                if agg_mode == "dve":
                    for t in range(2):
                        at = agg_pool.tile([128, GO], BF16, name=f"aggT{b}_{t}",
                                           tag=f"agg{t}")
                        acc = acc_pool.tile([128, GO], F32, name=f"acc{b}_{t}",
                                            tag="acca")
                        nc.vector.tensor_scalar_mul(acc[:], w6_sb[t][:, 0:GO],
                                                    knb4[:, 0:1])
                        for k in range(1, 4):
                            nc.vector.scalar_tensor_tensor(
                                acc[:], w6_sb[t][:, k * GO : (k + 1) * GO],
                                knb4[:, k : k + 1], acc[:],
                                op0=mybir.AluOpType.mult, op1=mybir.AluOpType.add)
                        for ij in range(9):
                            nc.vector.tensor_scalar_mul(
                                at[:, ij * 256 : (ij + 1) * 256],
                                acc[:, ij * 256 : (ij + 1) * 256],
                                chsp[:, 9 * t + ij : 9 * t + ij + 1])
                        aggT.append(at)
                    st["aggT"] = aggT
                    state[b] = st
                    return
                bounds = [(0, 512), (512, 1024), (1024, 1536), (1536, 2048),
                          (2048, 2304)]
                for t in range(2):
                    at = agg_pool.tile([128, GO], BF16, name=f"aggT{b}_{t}",
                                       tag=f"agg{t}")
                    for (c0, c1) in bounds:
                        n = c1 - c0
                        aps = aps_pool.tile([128, 512], F32,
                                            name=f"aps{b}_{t}_{c0}", tag="aps")
                        # 16 concurrent 32x32 tiles: row-group i holds expert
                        # i's weights, col-group j accumulates channel group
                        # 4t+j; stripe[32i:32i+32] is kn[i]*I32.
                        for j in range(4):
                            g0 = (4 * t + j) * GO
                            if agg_tiled:
                                for i in range(4):
                                    nc.tensor.matmul(
                                        aps[32 * j : 32 * j + 32, 0:n],
                                        stripe[32 * i : 32 * i + 32, :],
                                        w5_sb[32 * i : 32 * i + 32, g0 + c0 : g0 + c1],
                                        tile_position=(32 * i, 32 * j),
                                        start=(i == 0), stop=(i == 3),
                                    )
                            else:
                                nc.tensor.matmul(
                                    aps[32 * j : 32 * j + 32, 0:n],
                                    stripe[:],
                                    w5_sb[:, g0 + c0 : g0 + c1],
                                    tile_position=(0, 32 * j),
                                )
                        for ij in range(c0 // 256, c1 // 256):
                            nc.vector.tensor_scalar_mul(
                                at[:, ij * 256 : (ij + 1) * 256],
                                aps[:, ij * 256 - c0 : (ij + 1) * 256 - c0],
                                chsp[:, 9 * t + ij : 9 * t + ij + 1],
                            )
                    aggT.append(at)
                st["aggT"] = aggT
                state[b] = st

            def conv(b):
                st = state[b]
                xv = [st["xp"][t][:].rearrange("p (h w) -> p h w", w=PW)
                      for t in range(2)]
                for ot in range(2):
                    for nt in range(NT):
                        cps = cps_pool.tile([128, NFREE], F32,
                                            name=f"cps{b}_{ot}_{nt}", tag="cps")
                        idx = 0
                        for t in range(2):
                            for ij in range(9):
                                i, jj = divmod(ij, 3)
                                nc.tensor.matmul(
                                    cps[:],
                                    st["aggT"][t][:, ij * 256 + ot * 128 :
                                                  ij * 256 + ot * 128 + 128],
                                    xv[t][:, ROWS * nt + i : ROWS * nt + i + ROWS,
                                          jj : jj + W],
                                    start=(idx == 0), stop=(idx == 17),
                                )
                                idx += 1
                        osb = osb_pool.tile([128, NFREE], F32,
                                            name=f"osb{b}_{ot}_{nt}", tag="osb")
                        nc.scalar.activation(osb[:], cps[:], AF.Copy,
                                             scale=st["fl"][:, ot : ot + 1])
                        nc.sync.dma_start(
                            out[b * C + ot * 128 : b * C + ot * 128 + 128,
                                nt * NFREE : (nt + 1) * NFREE],
                            osb[:],
                        )
                del state[b]

            def body():
                for b in range(depth):
                    prep(b)
                for b in range(depth, nb):
                    prep(b)
                    conv(b - depth)
                for b in range(nb - depth, nb):
                    conv(b)

            if loop_r is None:
                body()
            else:
                with tc.For_i(0, loop_r, 1):
                    body()

    if not nc.is_finalized():
        nc.finalize()
    return nc


_NC_CACHE = None


def _get_nc(loop_r=None):
    global _NC_CACHE
    if loop_r is not None:
        return _build_nc(loop_r)
    if _NC_CACHE is None:
        _NC_CACHE = _build_nc()
    return _NC_CACHE


def _host_prep(x, weight, fc_w, bn_gamma, bn_beta, ch_w, ch_b, fl_w, fl_b,
               sp_w, sp_b, kn_w, kn_b):
    import ml_dtypes
    f = np.float32
    bf = ml_dtypes.bfloat16

    x = np.ascontiguousarray(x, dtype=f)
    xpad = np.zeros((B, C, PH, PW), dtype=bf)
    xpad[:, :, 1:-1, 1:-1] = x.astype(bf)
    xpad = xpad.reshape(B, C, PHW)

    # W5[(k,c32), g, ij*O+o] = weight[k, o, g*32+c32, ij]
    wt = np.asarray(weight, dtype=f).reshape(K, O, 8, 32, 9)  # k o g c32 ij
    w5 = np.ascontiguousarray(
        wt.transpose(0, 3, 2, 4, 1).reshape(128, 8 * GO).astype(bf)
    )
    # W6[c, k, ij*O+o] = weight[k, o, c, ij]
    w6 = np.ascontiguousarray(
        np.asarray(weight, dtype=f).reshape(K, O, C, 9)
        .transpose(2, 0, 3, 1).reshape(C, 4 * GO).astype(bf)
    )
    onesr = np.ones((1, 128), dtype=f)

    g16 = np.asarray(bn_gamma, dtype=f) / np.sqrt(f(1.0) + f(EPS))
    fc_w2 = (np.asarray(fc_w, dtype=f) * g16[:, None] / f(HW)).T  # [256,16]
    fcw = np.ascontiguousarray(np.concatenate([fc_w2[:128], fc_w2[128:]], axis=1))

    it = f(1.0 / TEMP)
    headsw = np.zeros((16, 269), dtype=f)
    headsw[:, 0:256] = np.asarray(ch_w, f).T * it
    headsw[:, 256:265] = np.asarray(sp_w, f).T * it
    headsw[:, 265:269] = np.asarray(kn_w, f).T * it
    flw = np.ascontiguousarray(np.asarray(fl_w, f).T * it)

    brow = np.zeros((1, BROW_COLS), dtype=f)
    brow[0, BB_BETA : BB_BETA + 16] = np.asarray(bn_beta, f)
    brow[0, BB_HEAD : BB_HEAD + 256] = np.asarray(ch_b, f) * it
    brow[0, BB_HEAD + 256 : BB_HEAD + 265] = np.asarray(sp_b, f) * it
    brow[0, BB_HEAD + 265 : BB_HEAD + 269] = np.asarray(kn_b, f) * it
    brow[0, BB_FL : BB_FL + 256] = np.asarray(fl_b, f) * it

    cmask = np.ascontiguousarray(np.tile(np.eye(32, dtype=f), (4, 1)))
    cp4 = np.ascontiguousarray(np.repeat(np.eye(4, dtype=f), 32, axis=1))
    ones4 = np.ones((4, 1), dtype=f)

    shared = dict(w5=w5, w6=w6, onesr=onesr, fcw=fcw, headsw=headsw,
                  flw=flw, brow=brow, cmask=cmask, cp4=cp4, ones4=ones4)
    in_maps = []
    for ci in range(NCORES):
        m = dict(shared)
        m["xpad"] = np.ascontiguousarray(
            xpad[ci * BL : (ci + 1) * BL].reshape(BL * C, PHW)
        )
        in_maps.append(m)
    return in_maps


def kernel(**inputs):
    nc = _get_nc()
    in_maps = _host_prep(**inputs)
    res = run_bass_kernel_spmd(nc, in_maps, list(range(NCORES)))
    outs = [res.results[i]["out"].reshape(BL, C, H, W) for i in range(NCORES)]
    return np.concatenate(outs, axis=0)


if __name__ == "__main__":
    nc = _get_nc()
    print("built ok")
